# revision 1
# baseline (speedup 1.0000x reference)
"""Trainium2 Bass kernel for the compositional skeleton loss.

loss = mean_b sum_{pairs p, xyz c} | (C @ bones_in)[b,p,c] - (T @ bones_tgt)[b,p,c] |

Reformulated as one matmul per batch row:  delta_row = z_row @ W, where
z_row = [input_row (63), target_row (63)] and W is [126, 630] built from the
signed path-sum matrix C and the endpoint-diff matrix T (block structure over
the 3 xyz channels), followed by abs + total sum, / B.

Sharding: pure data parallel over the batch dim across 8 NeuronCores.
Each core returns per-partition partial sums [126,1]; host adds them up.
"""

import numpy as np
from collections import deque
from itertools import combinations

# ---------------------------------------------------------------- constants
NJ = 21
B_FULL = 65536
N_CORES = 8
B_CORE = B_FULL // N_CORES  # 8192

_JOINTS = ['Ab', 'Chest', 'Head', 'Hip', 'LFArm', 'LFoot', 'LHand', 'LShin',
           'LShoulder', 'LThigh', 'LToe', 'LUArm', 'Neck', 'RFArm', 'RFoot',
           'RHand', 'RShin', 'RShoulder', 'RThigh', 'RToe', 'RUArm']
_PARENTS = {'Ab': 'Hip', 'Chest': 'Ab', 'Head': 'Neck', 'Hip': 'Hip',
            'LFArm': 'LUArm', 'LFoot': 'LShin', 'LHand': 'LFArm',
            'LShin': 'LThigh', 'LShoulder': 'Chest', 'LThigh': 'Hip',
            'LToe': 'LFoot', 'LUArm': 'LShoulder', 'Neck': 'Chest',
            'RFArm': 'RUArm', 'RFoot': 'RShin', 'RHand': 'RFArm',
            'RShin': 'RThigh', 'RShoulder': 'Chest', 'RThigh': 'Hip',
            'RToe': 'RFoot', 'RUArm': 'RShoulder'}


def _build_w():
    idx = {n: i for i, n in enumerate(_JOINTS)}
    par = {idx[k]: idx[v] for k, v in _PARENTS.items()}
    adj = {j: [] for j in range(NJ)}
    for j, p in par.items():
        if j != p:
            adj[j].append(p)
            adj[p].append(j)

    def bfs_path(u, v):
        prev = {u: None}
        q = deque([u])
        while q:
            x = q.popleft()
            if x == v:
                break
            for y in adj[x]:
                if y not in prev:
                    prev[y] = x
                    q.append(y)
        path = [v]
        while prev[path[-1]] is not None:
            path.append(prev[path[-1]])
        return path[::-1]

    pairs = list(combinations(range(NJ), 2))  # 210
    c_np = np.zeros((len(pairs), NJ), np.float32)
    t_np = np.zeros((len(pairs), NJ), np.float32)
    for pi, (u, v) in enumerate(pairs):
        pa = bfs_path(u, v)
        for m in range(len(pa) - 1):
            c_np[pi, pa[m]] += 1.0 if par[pa[m]] == pa[m + 1] else -1.0
        t_np[pi, u] += 1.0
        t_np[pi, v] -= 1.0

    # W[t*63 + j*3 + c, p*3 + c] = C[p,j] (t=0) / -T[p,j] (t=1)
    eye3 = np.eye(3, dtype=np.float32)
    w_in = np.einsum('pj,cd->jcpd', c_np, eye3).reshape(63, 630)
    w_tg = np.einsum('pj,cd->jcpd', -t_np, eye3).reshape(63, 630)
    return np.ascontiguousarray(np.concatenate([w_in, w_tg], axis=0))  # [126, 630]


_W = _build_w()

# ---------------------------------------------------------------- bass build
R_PER_GRP = 8                       # 128-row tiles per group
N_GRP = B_CORE // (128 * R_PER_GRP)  # 8
N_CCH = 5                           # 630 = 5 x 126 output-column chunks

_NC = None


def _build_bass(n_reps=1):
    import concourse.bacc as bacc
    import concourse.mybir as mybir
    import concourse.tile as tile

    f32 = mybir.dt.float32
    nc = bacc.Bacc("TRN2", target_bir_lowering=False, debug=False)

    x = nc.dram_tensor("x", [B_CORE, 63], f32, kind="ExternalInput")
    y = nc.dram_tensor("y", [B_CORE, 63], f32, kind="ExternalInput")
    out = nc.dram_tensor("out", [126, 1], f32, kind="ExternalOutput")

    w_dram = nc.inline_tensor(_W, name="w_const")
    ident_dram = nc.inline_tensor(np.eye(128, dtype=np.float32), name="ident_const")

    with tile.TileContext(nc) as tc:
        with (
            tc.tile_pool(name="consts", bufs=1) as consts,
            tc.tile_pool(name="staged", bufs=4) as staged_pool,
            tc.tile_pool(name="zt", bufs=3) as zt_pool,
            tc.tile_pool(name="psumT", bufs=2, space="PSUM") as psumT_pool,
            tc.tile_pool(name="psumD", bufs=3, space="PSUM") as psumD_pool,
            tc.tile_pool(name="misc", bufs=1) as misc,
        ):
            w_sb = consts.tile([126, 630], f32)
            nc.sync.dma_start(w_sb[:], w_dram[:])
            id_sb = consts.tile([128, 128], f32)
            nc.sync.dma_start(id_sb[:], ident_dram[:])

            scratch = misc.tile([126, 1024], f32)  # ACT abs dump (never read)

            # greedy ACT/DVE balance (ns-per-op estimates incl. errata)
            eng_t = {"act": 0.0, "dve": 0.0}

            def pick_engine(act_ns, dve_ns):
                e = "act" if eng_t["act"] + act_ns <= \
                    eng_t["dve"] + dve_ns else "dve"
                eng_t[e] += act_ns if e == "act" else dve_ns
                return e

            def emit_copy(dst, src):
                if pick_engine(570.0, 658.0) == "act":
                    nc.scalar.copy(dst, src)
                else:
                    nc.vector.tensor_copy(dst, src)

            def emit_absred(col, dps):
                if pick_engine(997.0, 1192.0) == "act":
                    nc.scalar.activation(
                        scratch[:], dps[:],
                        mybir.ActivationFunctionType.Abs, accum_out=col)
                else:
                    nc.vector.tensor_reduce(
                        col, dps[:], axis=mybir.AxisListType.X,
                        op=mybir.AluOpType.add, apply_absolute_value=True)

            rows = 128 * R_PER_GRP  # 1024

            for rep in range(n_reps):
                acc = staged_pool.tile([126, N_GRP * N_CCH], f32, tag="acc")
                final = staged_pool.tile([126, 1], f32, tag="final")

                prev = None  # (zt, g) pending matmul+epilogue

                def flush_prev():
                    zt, g = prev
                    for c in range(N_CCH):
                        # two matmuls fill a 2-bank psum tile; one 1024-wide
                        # fused abs+sum drains it
                        dps = psumD_pool.tile([126, 1024], f32)
                        nc.tensor.matmul(
                            dps[:, 0:512],
                            w_sb[:, c * 126:(c + 1) * 126], zt[:, 0:512])
                        nc.tensor.matmul(
                            dps[:, 512:1024],
                            w_sb[:, c * 126:(c + 1) * 126], zt[:, 512:1024])
                        emit_absred(
                            acc[:, g * N_CCH + c: g * N_CCH + c + 1], dps)

                for g in range(N_GRP):
                    st = staged_pool.tile([128, R_PER_GRP, 126], f32)
                    xv = x[g * rows:(g + 1) * rows, :].rearrange(
                        "(p r) j -> p r j", p=128)
                    yv = y[g * rows:(g + 1) * rows, :].rearrange(
                        "(p r) j -> p r j", p=128)
                    nc.sync.dma_start(st[:, :, 0:63], xv)
                    nc.sync.dma_start(st[:, :, 63:126], yv)

                    # transpose 8x [128,126] -> two [126,512] psum tiles
                    zt = zt_pool.tile([126, 1024], f32)
                    for h in range(2):
                        zt_ps = psumT_pool.tile([126, 512], f32)
                        for r in range(4):
                            nc.tensor.transpose(
                                zt_ps[:, r * 128:(r + 1) * 128],
                                st[:, h * 4 + r, :], id_sb[:])
                        emit_copy(zt[:, h * 512:(h + 1) * 512], zt_ps[:])

                    # software pipeline: matmuls for the PREVIOUS group run
                    # after this group's transposes, so the PE never waits
                    # on the psum->sbuf copy of its rhs
                    if prev is not None:
                        flush_prev()
                    prev = (zt, g)

                flush_prev()

                nc.vector.tensor_reduce(
                    final[:], acc[:], axis=mybir.AxisListType.X,
                    op=mybir.AluOpType.add)
                nc.sync.dma_start(out[:], final[:])

    nc.compile()
    return nc


def kernel(input, target):
    global _NC
    from concourse.bass_utils import run_bass_kernel_spmd

    if _NC is None:
        _NC = _build_bass()

    inp = np.ascontiguousarray(np.asarray(input, dtype=np.float32))
    tgt = np.ascontiguousarray(np.asarray(target, dtype=np.float32))
    assert inp.shape == (B_FULL, NJ * 3) and tgt.shape == (B_FULL, NJ * 3)

    in_maps = []
    for i in range(N_CORES):
        sl = slice(i * B_CORE, (i + 1) * B_CORE)
        in_maps.append({
            "x": np.ascontiguousarray(inp[sl]),
            "y": np.ascontiguousarray(tgt[sl]),
        })

    res = run_bass_kernel_spmd(_NC, in_maps, core_ids=list(range(N_CORES)))
    total = np.float64(0.0)
    for r in res.results:
        total += np.float64(r["out"].astype(np.float64).sum())
    return np.array([total / B_FULL], dtype=np.float32)



# revision 3
# speedup vs baseline: 515.1366x; 515.1366x over previous
"""Trainium2 Bass kernel for the compositional skeleton loss.

loss = mean_b sum_{pairs p, xyz c} | (C @ bones_in)[b,p,c] - (T @ bones_tgt)[b,p,c] |

Reformulated as one matmul per batch row:  delta_row = z_row @ W, where
z_row = [input_row (63), target_row (63)] and W is [126, 630] built from the
signed path-sum matrix C and the endpoint-diff matrix T (block structure over
the 3 xyz channels), followed by abs + total sum, / B.

Device pipeline (per core, pure batch-parallel across 8 NeuronCores):
  - host pre-transposes and casts:  zt = z.T as bf16 [126, B_CORE]  (so no
    on-device transposes or PSUM->SBUF copies are needed; W entries are
    exactly representable in bf16, z rounding is ~0.4% << 2e-2 tolerance)
  - PE: delta chunks [128, 2048] = Wc.T @ zt   (5 column chunks of W, padded
    126->128 outputs each)
  - ACT/DVE: fused abs+sum of each PSUM tile -> acc column (the engine wall:
    both engines read PSUM at 1 elem/lane/cycle, so they are statically
    load-balanced ~11:9)
  - final column reduce -> out [128, 1]; host sums across cores / B.
"""

import numpy as np
from collections import deque
from itertools import combinations

# ---------------------------------------------------------------- constants
NJ = 21
B_FULL = 65536
N_CORES = 8
B_CORE = B_FULL // N_CORES  # 8192

N_CCH = 5            # 630 outputs = 5 chunks of 126 (each padded to 128)
FD = 2048            # PSUM tile free dim (4 banks fp32)
N_WIN = B_CORE // FD  # 4 windows per rep

_JOINTS = ['Ab', 'Chest', 'Head', 'Hip', 'LFArm', 'LFoot', 'LHand', 'LShin',
           'LShoulder', 'LThigh', 'LToe', 'LUArm', 'Neck', 'RFArm', 'RFoot',
           'RHand', 'RShin', 'RShoulder', 'RThigh', 'RToe', 'RUArm']
_PARENTS = {'Ab': 'Hip', 'Chest': 'Ab', 'Head': 'Neck', 'Hip': 'Hip',
            'LFArm': 'LUArm', 'LFoot': 'LShin', 'LHand': 'LFArm',
            'LShin': 'LThigh', 'LShoulder': 'Chest', 'LThigh': 'Hip',
            'LToe': 'LFoot', 'LUArm': 'LShoulder', 'Neck': 'Chest',
            'RFArm': 'RUArm', 'RFoot': 'RShin', 'RHand': 'RFArm',
            'RShin': 'RThigh', 'RShoulder': 'Chest', 'RThigh': 'Hip',
            'RToe': 'RFoot', 'RUArm': 'RShoulder'}


def _build_w():
    idx = {n: i for i, n in enumerate(_JOINTS)}
    par = {idx[k]: idx[v] for k, v in _PARENTS.items()}
    adj = {j: [] for j in range(NJ)}
    for j, p in par.items():
        if j != p:
            adj[j].append(p)
            adj[p].append(j)

    def bfs_path(u, v):
        prev = {u: None}
        q = deque([u])
        while q:
            x = q.popleft()
            if x == v:
                break
            for y in adj[x]:
                if y not in prev:
                    prev[y] = x
                    q.append(y)
        path = [v]
        while prev[path[-1]] is not None:
            path.append(prev[path[-1]])
        return path[::-1]

    pairs = list(combinations(range(NJ), 2))  # 210
    c_np = np.zeros((len(pairs), NJ), np.float32)
    t_np = np.zeros((len(pairs), NJ), np.float32)
    for pi, (u, v) in enumerate(pairs):
        pa = bfs_path(u, v)
        for m in range(len(pa) - 1):
            c_np[pi, pa[m]] += 1.0 if par[pa[m]] == pa[m + 1] else -1.0
        t_np[pi, u] += 1.0
        t_np[pi, v] -= 1.0

    # W[t*63 + j*3 + c, p*3 + c] = C[p,j] (t=0) / -T[p,j] (t=1)
    eye3 = np.eye(3, dtype=np.float32)
    w_in = np.einsum('pj,cd->jcpd', c_np, eye3).reshape(63, 630)
    w_tg = np.einsum('pj,cd->jcpd', -t_np, eye3).reshape(63, 630)
    return np.ascontiguousarray(np.concatenate([w_in, w_tg], axis=0))  # [126, 630]


_W = _build_w()


def _build_w_padded():
    """[126, 640] bf16: chunk c of W's columns at [128c, 128c+126), zero pad."""
    import ml_dtypes
    wp = np.zeros((126, N_CCH * 128), np.float32)
    for c in range(N_CCH):
        wp[:, 128 * c:128 * c + 126] = _W[:, 126 * c:126 * (c + 1)]
    return wp.astype(ml_dtypes.bfloat16)


# ---------------------------------------------------------------- bass build
_NC = None


def _build_bass(n_reps=1):
    import concourse.bacc as bacc
    import concourse.mybir as mybir
    import concourse.tile as tile

    f32 = mybir.dt.float32
    bf16 = mybir.dt.bfloat16
    nc = bacc.Bacc("TRN2", target_bir_lowering=False, debug=False)

    zt = nc.dram_tensor("zt", [126, B_CORE], bf16, kind="ExternalInput")
    out = nc.dram_tensor("out", [128, 1], f32, kind="ExternalOutput")

    w_dram = nc.inline_tensor(_build_w_padded(), name="w_const")

    with tile.TileContext(nc) as tc:
        with (
            tc.tile_pool(name="consts", bufs=1) as consts,
            tc.tile_pool(name="ztp", bufs=2) as zt_pool,
            tc.tile_pool(name="staged", bufs=2) as staged_pool,
            tc.tile_pool(name="psumD", bufs=2, space="PSUM") as psum_pool,
            tc.tile_pool(name="misc", bufs=1) as misc,
        ):
            w_sb = consts.tile([126, N_CCH * 128], bf16)
            nc.sync.dma_start(w_sb[:], w_dram[:])

            scratch = misc.tile([128, FD], f32)  # ACT abs dump (never read)

            # greedy ACT/DVE balance (ns-per-op estimates incl. errata)
            eng_t = {"act": 0.0, "dve": 0.0}

            def emit_absred(col, dps):
                act_ns = (172.0 + FD) / 1.2
                dve_ns = (120.0 + FD) / 0.96
                if eng_t["act"] + act_ns <= eng_t["dve"] + dve_ns:
                    eng_t["act"] += act_ns
                    nc.scalar.activation(
                        scratch[:], dps[:],
                        mybir.ActivationFunctionType.Abs, accum_out=col)
                else:
                    eng_t["dve"] += dve_ns
                    nc.vector.tensor_reduce(
                        col, dps[:], axis=mybir.AxisListType.X,
                        op=mybir.AluOpType.add, apply_absolute_value=True)

            for rep in range(n_reps):
                zt_sb = zt_pool.tile([126, B_CORE], bf16)
                nc.sync.dma_start(zt_sb[:], zt[:])

                acc = staged_pool.tile([128, N_WIN * N_CCH], f32, tag="acc")
                final = staged_pool.tile([128, 1], f32, tag="final")

                for w in range(N_WIN):
                    for c in range(N_CCH):
                        dps = psum_pool.tile([128, FD], f32)
                        for q in range(FD // 512):
                            nc.tensor.matmul(
                                dps[:, q * 512:(q + 1) * 512],
                                w_sb[:, 128 * c:128 * c + 128],
                                zt_sb[:, w * FD + q * 512:w * FD + (q + 1) * 512])
                        i = w * N_CCH + c
                        emit_absred(acc[:, i:i + 1], dps)

                nc.vector.tensor_reduce(
                    final[:], acc[:], axis=mybir.AxisListType.X,
                    op=mybir.AluOpType.add)
                nc.sync.dma_start(out[:], final[:])

    nc.compile()
    return nc


# ---------------------------------------------------------------- host side
def _make_in_maps(inp, tgt):
    import ml_dtypes
    z = np.concatenate(
        [np.asarray(inp, np.float32), np.asarray(tgt, np.float32)], axis=1)
    in_maps = []
    for i in range(N_CORES):
        sl = slice(i * B_CORE, (i + 1) * B_CORE)
        zt_i = np.ascontiguousarray(z[sl].T).astype(ml_dtypes.bfloat16)
        in_maps.append({"zt": zt_i})
    return in_maps


def kernel(input, target):
    global _NC
    from concourse.bass_utils import run_bass_kernel_spmd

    if _NC is None:
        _NC = _build_bass()

    assert input.shape == (B_FULL, NJ * 3) and target.shape == (B_FULL, NJ * 3)
    in_maps = _make_in_maps(input, target)

    res = run_bass_kernel_spmd(_NC, in_maps, core_ids=list(range(N_CORES)))
    total = np.float64(0.0)
    for r in res.results:
        total += np.float64(r["out"].astype(np.float64).sum())
    return np.array([total / B_FULL], dtype=np.float32)


# revision 8
# speedup vs baseline: 720.8859x; 1.3994x over previous
"""Trainium2 Bass kernel for the compositional skeleton loss.

loss = mean_b sum_{pairs p, xyz c} | (C @ bones_in)[b,p,c] - (T @ bones_tgt)[b,p,c] |

Reformulated as one matmul per batch row:  delta_row = z_row @ W, where
z_row = [input_row (63), target_row (63)] and W is [126, 630] built from the
signed path-sum matrix C and the endpoint-diff matrix T (block structure over
the 3 xyz channels), followed by abs + total sum, / B.

Device pipeline (per core, pure batch-parallel across 8 NeuronCores):
  - host pre-transposes and casts:  zt = z.T as bf16 [126, B_CORE]  (so no
    on-device transposes or PSUM->SBUF copies are needed; W entries are
    exactly representable in bf16, z rounding is ~0.4% << 2e-2 tolerance)
  - PE: delta chunks [128, 2048] = Wc.T @ zt   (5 column chunks of W, padded
    126->128 outputs each)
  - ACT/DVE: fused abs+sum of each PSUM tile -> acc column (the engine wall:
    both engines read PSUM at 1 elem/lane/cycle, so they are statically
    load-balanced ~11:9)
  - final column reduce -> out [128, 1]; host sums across cores / B.
"""

import numpy as np
from collections import deque
from itertools import combinations

# ---------------------------------------------------------------- constants
NJ = 21
B_FULL = 65536
N_CORES = 8
B_CORE = B_FULL // N_CORES  # 8192

N_CCH = 5            # 630 outputs = 5 chunks of 126 (each padded to 128)

_JOINTS = ['Ab', 'Chest', 'Head', 'Hip', 'LFArm', 'LFoot', 'LHand', 'LShin',
           'LShoulder', 'LThigh', 'LToe', 'LUArm', 'Neck', 'RFArm', 'RFoot',
           'RHand', 'RShin', 'RShoulder', 'RThigh', 'RToe', 'RUArm']
_PARENTS = {'Ab': 'Hip', 'Chest': 'Ab', 'Head': 'Neck', 'Hip': 'Hip',
            'LFArm': 'LUArm', 'LFoot': 'LShin', 'LHand': 'LFArm',
            'LShin': 'LThigh', 'LShoulder': 'Chest', 'LThigh': 'Hip',
            'LToe': 'LFoot', 'LUArm': 'LShoulder', 'Neck': 'Chest',
            'RFArm': 'RUArm', 'RFoot': 'RShin', 'RHand': 'RFArm',
            'RShin': 'RThigh', 'RShoulder': 'Chest', 'RThigh': 'Hip',
            'RToe': 'RFoot', 'RUArm': 'RShoulder'}


def _build_w():
    idx = {n: i for i, n in enumerate(_JOINTS)}
    par = {idx[k]: idx[v] for k, v in _PARENTS.items()}
    adj = {j: [] for j in range(NJ)}
    for j, p in par.items():
        if j != p:
            adj[j].append(p)
            adj[p].append(j)

    def bfs_path(u, v):
        prev = {u: None}
        q = deque([u])
        while q:
            x = q.popleft()
            if x == v:
                break
            for y in adj[x]:
                if y not in prev:
                    prev[y] = x
                    q.append(y)
        path = [v]
        while prev[path[-1]] is not None:
            path.append(prev[path[-1]])
        return path[::-1]

    pairs = list(combinations(range(NJ), 2))  # 210
    c_np = np.zeros((len(pairs), NJ), np.float32)
    t_np = np.zeros((len(pairs), NJ), np.float32)
    for pi, (u, v) in enumerate(pairs):
        pa = bfs_path(u, v)
        for m in range(len(pa) - 1):
            c_np[pi, pa[m]] += 1.0 if par[pa[m]] == pa[m + 1] else -1.0
        t_np[pi, u] += 1.0
        t_np[pi, v] -= 1.0

    # W[t*63 + j*3 + c, p*3 + c] = C[p,j] (t=0) / -T[p,j] (t=1)
    eye3 = np.eye(3, dtype=np.float32)
    w_in = np.einsum('pj,cd->jcpd', c_np, eye3).reshape(63, 630)
    w_tg = np.einsum('pj,cd->jcpd', -t_np, eye3).reshape(63, 630)
    return np.ascontiguousarray(np.concatenate([w_in, w_tg], axis=0))  # [126, 630]


_W = _build_w()


def _build_w_padded():
    """[126, 640] bf16: chunk c of W's columns at [128c, 128c+126), zero pad."""
    import ml_dtypes
    wp = np.zeros((126, N_CCH * 128), np.float32)
    for c in range(N_CCH):
        wp[:, 128 * c:128 * c + 126] = _W[:, 126 * c:126 * (c + 1)]
    return wp.astype(ml_dtypes.bfloat16)


# ---------------------------------------------------------------- bass build
_NC = None

# tuning knobs (A/B'd on hardware)
CONFIG = {
    "act_ns": 1850.0,   # sustained ns per [128, FD] abs-reduce tile on ACT
    "dve_ns": 3000.0,   # sustained ns per tile on DVE (incl. DRAIN penalty)
    "split": "auto",    # "auto" greedy by the ns above | "act" | "dve"
    "n_act": None,      # int: exactly n_act of the tiles go to ACT (Bresenham
                        # interleaved with DVE); overrides "split"
    "psum_bf16": False,  # matmul writes bf16 PSUM (2 banks/tile, 1024-col MMs)
    "psum_bufs": 2,
    "fd": 2048,         # PSUM tile free dim
}


def _build_bass(n_reps=1):
    import concourse.bacc as bacc
    import concourse.mybir as mybir
    import concourse.tile as tile

    f32 = mybir.dt.float32
    bf16 = mybir.dt.bfloat16
    nc = bacc.Bacc("TRN2", target_bir_lowering=False, debug=False)

    zt = nc.dram_tensor("zt", [126, B_CORE], bf16, kind="ExternalInput")
    out = nc.dram_tensor("out", [128, 1], f32, kind="ExternalOutput")

    w_dram = nc.inline_tensor(_build_w_padded(), name="w_const")

    psum_dt = bf16 if CONFIG["psum_bf16"] else f32
    mm_n = 1024 if CONFIG["psum_bf16"] else 512  # one PSUM bank per matmul
    FD = CONFIG["fd"]
    N_WIN = B_CORE // FD
    n_tiles = N_WIN * N_CCH

    with tile.TileContext(nc) as tc:
        with (
            tc.tile_pool(name="consts", bufs=1) as consts,
            tc.tile_pool(name="ztp", bufs=2) as zt_pool,
            tc.tile_pool(name="staged", bufs=2) as staged_pool,
            tc.tile_pool(name="psumD", bufs=CONFIG["psum_bufs"],
                         space="PSUM") as psum_pool,
            tc.tile_pool(name="misc", bufs=1) as misc,
        ):
            w_sb = consts.tile([126, N_CCH * 128], bf16)
            nc.sync.dma_start(w_sb[:], w_dram[:])

            scratch = misc.tile([128, FD], psum_dt)  # ACT abs dump (never read)

            # greedy ACT/DVE balance (sustained-ns estimates)
            eng_t = {"act": 0.0, "dve": 0.0, "i": 0}

            def emit_absred(col, dps):
                act_ns, dve_ns = CONFIG["act_ns"], CONFIG["dve_ns"]
                i = eng_t["i"] % n_tiles
                eng_t["i"] += 1
                if CONFIG["n_act"] is not None:
                    na = CONFIG["n_act"]
                    use_act = ((i + 1) * na) // n_tiles > (i * na) // n_tiles
                elif CONFIG["split"] == "act":
                    use_act = True
                elif CONFIG["split"] == "dve":
                    use_act = False
                else:
                    use_act = eng_t["act"] + act_ns <= eng_t["dve"] + dve_ns
                if use_act:
                    eng_t["act"] += act_ns
                    nc.scalar.activation(
                        scratch[:], dps[:],
                        mybir.ActivationFunctionType.Abs, accum_out=col)
                else:
                    eng_t["dve"] += dve_ns
                    nc.vector.tensor_reduce(
                        col, dps[:], axis=mybir.AxisListType.X,
                        op=mybir.AluOpType.add, apply_absolute_value=True)

            for rep in range(n_reps):
                zt_sb = zt_pool.tile([126, B_CORE], bf16)
                nc.sync.dma_start(zt_sb[:], zt[:])

                acc = staged_pool.tile([128, N_WIN * N_CCH], f32, tag="acc")
                final = staged_pool.tile([128, 1], f32, tag="final")

                for w in range(N_WIN):
                    for c in range(N_CCH):
                        dps = psum_pool.tile([128, FD], psum_dt)
                        for q in range(FD // mm_n):
                            nc.tensor.matmul(
                                dps[:, q * mm_n:(q + 1) * mm_n],
                                w_sb[:, 128 * c:128 * c + 128],
                                zt_sb[:, w * FD + q * mm_n:w * FD + (q + 1) * mm_n])
                        i = w * N_CCH + c
                        emit_absred(acc[:, i:i + 1], dps)

                nc.vector.tensor_reduce(
                    final[:], acc[:], axis=mybir.AxisListType.X,
                    op=mybir.AluOpType.add)
                nc.sync.dma_start(out[:], final[:])

    nc.compile()
    return nc


# ---------------------------------------------------------------- host side
def _make_in_maps(inp, tgt):
    import ml_dtypes
    z = np.concatenate(
        [np.asarray(inp, np.float32), np.asarray(tgt, np.float32)], axis=1)
    in_maps = []
    for i in range(N_CORES):
        sl = slice(i * B_CORE, (i + 1) * B_CORE)
        zt_i = np.ascontiguousarray(z[sl].T).astype(ml_dtypes.bfloat16)
        in_maps.append({"zt": zt_i})
    return in_maps


def kernel(input, target):
    global _NC
    from concourse.bass_utils import run_bass_kernel_spmd

    if _NC is None:
        _NC = _build_bass()

    assert input.shape == (B_FULL, NJ * 3) and target.shape == (B_FULL, NJ * 3)
    in_maps = _make_in_maps(input, target)

    res = run_bass_kernel_spmd(_NC, in_maps, core_ids=list(range(N_CORES)))
    total = np.float64(0.0)
    for r in res.results:
        total += np.float64(r["out"].astype(np.float64).sum())
    return np.array([total / B_FULL], dtype=np.float32)


# revision 9
# speedup vs baseline: 726.4918x; 1.0078x over previous
"""Trainium2 Bass kernel for the compositional skeleton loss.

loss = mean_b sum_{pairs p, xyz c} | (C @ bones_in)[b,p,c] - (T @ bones_tgt)[b,p,c] |

Reformulated as one matmul per batch row:  delta_row = z_row @ W, where
z_row = [input_row (63), target_row (63)] and W is [126, 630] built from the
signed path-sum matrix C and the endpoint-diff matrix T (block structure over
the 3 xyz channels), followed by abs + total sum, / B.

Device pipeline (per core, pure batch-parallel across 8 NeuronCores):
  - host pre-transposes and casts:  zt = z.T as bf16 [126, B_CORE]  (so no
    on-device transposes or PSUM->SBUF copies are needed; W entries are
    exactly representable in bf16, z rounding is ~0.4% << 2e-2 tolerance)
  - PE: delta chunks [128, 2048] = Wc.T @ zt   (5 column chunks of W, padded
    126->128 outputs each)
  - ACT/DVE: fused abs+sum of each PSUM tile -> acc column (the engine wall:
    both engines read PSUM at 1 elem/lane/cycle, so they are statically
    load-balanced ~11:9)
  - final column reduce -> out [128, 1]; host sums across cores / B.
"""

import numpy as np
from collections import deque
from itertools import combinations

# ---------------------------------------------------------------- constants
NJ = 21
B_FULL = 65536
N_CORES = 8
B_CORE = B_FULL // N_CORES  # 8192

N_CCH = 5            # 630 outputs = 5 chunks of 126 (each padded to 128)

_JOINTS = ['Ab', 'Chest', 'Head', 'Hip', 'LFArm', 'LFoot', 'LHand', 'LShin',
           'LShoulder', 'LThigh', 'LToe', 'LUArm', 'Neck', 'RFArm', 'RFoot',
           'RHand', 'RShin', 'RShoulder', 'RThigh', 'RToe', 'RUArm']
_PARENTS = {'Ab': 'Hip', 'Chest': 'Ab', 'Head': 'Neck', 'Hip': 'Hip',
            'LFArm': 'LUArm', 'LFoot': 'LShin', 'LHand': 'LFArm',
            'LShin': 'LThigh', 'LShoulder': 'Chest', 'LThigh': 'Hip',
            'LToe': 'LFoot', 'LUArm': 'LShoulder', 'Neck': 'Chest',
            'RFArm': 'RUArm', 'RFoot': 'RShin', 'RHand': 'RFArm',
            'RShin': 'RThigh', 'RShoulder': 'Chest', 'RThigh': 'Hip',
            'RToe': 'RFoot', 'RUArm': 'RShoulder'}


def _build_w():
    idx = {n: i for i, n in enumerate(_JOINTS)}
    par = {idx[k]: idx[v] for k, v in _PARENTS.items()}
    adj = {j: [] for j in range(NJ)}
    for j, p in par.items():
        if j != p:
            adj[j].append(p)
            adj[p].append(j)

    def bfs_path(u, v):
        prev = {u: None}
        q = deque([u])
        while q:
            x = q.popleft()
            if x == v:
                break
            for y in adj[x]:
                if y not in prev:
                    prev[y] = x
                    q.append(y)
        path = [v]
        while prev[path[-1]] is not None:
            path.append(prev[path[-1]])
        return path[::-1]

    pairs = list(combinations(range(NJ), 2))  # 210
    c_np = np.zeros((len(pairs), NJ), np.float32)
    t_np = np.zeros((len(pairs), NJ), np.float32)
    for pi, (u, v) in enumerate(pairs):
        pa = bfs_path(u, v)
        for m in range(len(pa) - 1):
            c_np[pi, pa[m]] += 1.0 if par[pa[m]] == pa[m + 1] else -1.0
        t_np[pi, u] += 1.0
        t_np[pi, v] -= 1.0

    # W[t*63 + j*3 + c, p*3 + c] = C[p,j] (t=0) / -T[p,j] (t=1)
    eye3 = np.eye(3, dtype=np.float32)
    w_in = np.einsum('pj,cd->jcpd', c_np, eye3).reshape(63, 630)
    w_tg = np.einsum('pj,cd->jcpd', -t_np, eye3).reshape(63, 630)
    return np.ascontiguousarray(np.concatenate([w_in, w_tg], axis=0))  # [126, 630]


_W = _build_w()


def _build_w_padded():
    """[126, 640] bf16: chunk c of W's columns at [128c, 128c+126), zero pad."""
    import ml_dtypes
    wp = np.zeros((126, N_CCH * 128), np.float32)
    for c in range(N_CCH):
        wp[:, 128 * c:128 * c + 126] = _W[:, 126 * c:126 * (c + 1)]
    return wp.astype(ml_dtypes.bfloat16)


# ---------------------------------------------------------------- bass build
_NC = None

# tuning knobs (A/B'd on hardware).  Winning config: fd=1024 x 4 PSUM bufs
# decouples matmul fills from the ACT/DVE PSUM reads (fd=2048 x 2 bufs lost
# ~6 us/rep to fill/reduce serialization); the kernel then sits on the
# PSUM-read wall itself (~22 us/rep, the no-matmul probe measures the same).
CONFIG = {
    "act_ns": 997.0,    # sustained ns per [128, FD] abs-reduce tile on ACT
    "dve_ns": 1192.0,   # sustained ns per tile on DVE
    "split": "auto",    # "auto" greedy by the ns above | "act" | "dve"
    "n_act": None,      # int: exactly n_act of the tiles go to ACT (Bresenham
                        # interleaved with DVE); overrides "split"
    "psum_bf16": False,  # matmul writes bf16 PSUM (rejected by bass: fp32 only)
    "psum_bufs": 4,
    "fd": 1024,         # PSUM tile free dim (2 banks)
}


def _build_bass(n_reps=1):
    import concourse.bacc as bacc
    import concourse.mybir as mybir
    import concourse.tile as tile

    f32 = mybir.dt.float32
    bf16 = mybir.dt.bfloat16
    nc = bacc.Bacc("TRN2", target_bir_lowering=False, debug=False)

    zt = nc.dram_tensor("zt", [126, B_CORE], bf16, kind="ExternalInput")
    out = nc.dram_tensor("out", [128, 1], f32, kind="ExternalOutput")

    w_dram = nc.inline_tensor(_build_w_padded(), name="w_const")

    psum_dt = bf16 if CONFIG["psum_bf16"] else f32
    mm_n = 1024 if CONFIG["psum_bf16"] else 512  # one PSUM bank per matmul
    FD = CONFIG["fd"]
    N_WIN = B_CORE // FD
    n_tiles = N_WIN * N_CCH

    with tile.TileContext(nc) as tc:
        with (
            tc.tile_pool(name="consts", bufs=1) as consts,
            tc.tile_pool(name="ztp", bufs=2) as zt_pool,
            tc.tile_pool(name="staged", bufs=2) as staged_pool,
            tc.tile_pool(name="psumD", bufs=CONFIG["psum_bufs"],
                         space="PSUM") as psum_pool,
            tc.tile_pool(name="misc", bufs=1) as misc,
        ):
            w_sb = consts.tile([126, N_CCH * 128], bf16)
            nc.sync.dma_start(w_sb[:], w_dram[:])

            scratch = misc.tile([128, FD], psum_dt)  # ACT abs dump (never read)

            # greedy ACT/DVE balance (sustained-ns estimates)
            eng_t = {"act": 0.0, "dve": 0.0, "i": 0}

            def emit_absred(col, dps):
                act_ns, dve_ns = CONFIG["act_ns"], CONFIG["dve_ns"]
                i = eng_t["i"] % n_tiles
                eng_t["i"] += 1
                if CONFIG["n_act"] is not None:
                    na = CONFIG["n_act"]
                    use_act = ((i + 1) * na) // n_tiles > (i * na) // n_tiles
                elif CONFIG["split"] == "act":
                    use_act = True
                elif CONFIG["split"] == "dve":
                    use_act = False
                else:
                    use_act = eng_t["act"] + act_ns <= eng_t["dve"] + dve_ns
                if use_act:
                    eng_t["act"] += act_ns
                    nc.scalar.activation(
                        scratch[:], dps[:],
                        mybir.ActivationFunctionType.Abs, accum_out=col)
                else:
                    eng_t["dve"] += dve_ns
                    nc.vector.tensor_reduce(
                        col, dps[:], axis=mybir.AxisListType.X,
                        op=mybir.AluOpType.add, apply_absolute_value=True)

            for rep in range(n_reps):
                zt_sb = zt_pool.tile([126, B_CORE], bf16)
                nc.sync.dma_start(zt_sb[:], zt[:])

                acc = staged_pool.tile([128, N_WIN * N_CCH], f32, tag="acc")
                final = staged_pool.tile([128, 1], f32, tag="final")

                for w in range(N_WIN):
                    for c in range(N_CCH):
                        dps = psum_pool.tile([128, FD], psum_dt)
                        for q in range(FD // mm_n):
                            nc.tensor.matmul(
                                dps[:, q * mm_n:(q + 1) * mm_n],
                                w_sb[:, 128 * c:128 * c + 128],
                                zt_sb[:, w * FD + q * mm_n:w * FD + (q + 1) * mm_n])
                        i = w * N_CCH + c
                        emit_absred(acc[:, i:i + 1], dps)

                nc.vector.tensor_reduce(
                    final[:], acc[:], axis=mybir.AxisListType.X,
                    op=mybir.AluOpType.add)
                nc.sync.dma_start(out[:], final[:])

    nc.compile()
    return nc


# ---------------------------------------------------------------- host side
def _make_in_maps(inp, tgt):
    import ml_dtypes
    z = np.concatenate(
        [np.asarray(inp, np.float32), np.asarray(tgt, np.float32)], axis=1)
    in_maps = []
    for i in range(N_CORES):
        sl = slice(i * B_CORE, (i + 1) * B_CORE)
        zt_i = np.ascontiguousarray(z[sl].T).astype(ml_dtypes.bfloat16)
        in_maps.append({"zt": zt_i})
    return in_maps


def kernel(input, target):
    global _NC
    from concourse.bass_utils import run_bass_kernel_spmd

    if _NC is None:
        _NC = _build_bass()

    assert input.shape == (B_FULL, NJ * 3) and target.shape == (B_FULL, NJ * 3)
    in_maps = _make_in_maps(input, target)

    res = run_bass_kernel_spmd(_NC, in_maps, core_ids=list(range(N_CORES)))
    total = np.float64(0.0)
    for r in res.results:
        total += np.float64(r["out"].astype(np.float64).sum())
    return np.array([total / B_FULL], dtype=np.float32)


# revision 12
# speedup vs baseline: 790.1320x; 1.0876x over previous
"""Trainium2 Bass kernel for the compositional skeleton loss.

loss = mean_b sum_{pairs p, xyz c} | (C @ bones_in)[b,p,c] - (T @ bones_tgt)[b,p,c] |

Reformulated as one matmul per batch row:  delta_row = z_row @ W, where
z_row = [input_row (63), target_row (63)] and W is [126, 630] built from the
signed path-sum matrix C and the endpoint-diff matrix T (block structure over
the 3 xyz channels), followed by abs + total sum, / B.

Device pipeline (per core, pure batch-parallel across 8 NeuronCores):
  - host pre-transposes and casts:  zt = z.T as bf16 [126, B_CORE]  (so no
    on-device transposes or PSUM->SBUF copies are needed; W entries are
    exactly representable in bf16, z rounding is ~0.4% << 2e-2 tolerance)
  - PE: delta chunks [128, 2048] = Wc.T @ zt   (5 column chunks of W, padded
    126->128 outputs each)
  - ACT/DVE: fused abs+sum of each PSUM tile -> acc column (the engine wall:
    both engines read PSUM at 1 elem/lane/cycle, so they are statically
    load-balanced ~11:9)
  - final column reduce -> out [128, 1]; host sums across cores / B.
"""

import numpy as np
from collections import deque
from itertools import combinations

# ---------------------------------------------------------------- constants
NJ = 21
B_FULL = 65536
N_CORES = 8
B_CORE = B_FULL // N_CORES  # 8192

N_CCH = 5            # 630 outputs = 5 chunks of 126 (each padded to 128)

_JOINTS = ['Ab', 'Chest', 'Head', 'Hip', 'LFArm', 'LFoot', 'LHand', 'LShin',
           'LShoulder', 'LThigh', 'LToe', 'LUArm', 'Neck', 'RFArm', 'RFoot',
           'RHand', 'RShin', 'RShoulder', 'RThigh', 'RToe', 'RUArm']
_PARENTS = {'Ab': 'Hip', 'Chest': 'Ab', 'Head': 'Neck', 'Hip': 'Hip',
            'LFArm': 'LUArm', 'LFoot': 'LShin', 'LHand': 'LFArm',
            'LShin': 'LThigh', 'LShoulder': 'Chest', 'LThigh': 'Hip',
            'LToe': 'LFoot', 'LUArm': 'LShoulder', 'Neck': 'Chest',
            'RFArm': 'RUArm', 'RFoot': 'RShin', 'RHand': 'RFArm',
            'RShin': 'RThigh', 'RShoulder': 'Chest', 'RThigh': 'Hip',
            'RToe': 'RFoot', 'RUArm': 'RShoulder'}


def _build_w():
    idx = {n: i for i, n in enumerate(_JOINTS)}
    par = {idx[k]: idx[v] for k, v in _PARENTS.items()}
    adj = {j: [] for j in range(NJ)}
    for j, p in par.items():
        if j != p:
            adj[j].append(p)
            adj[p].append(j)

    def bfs_path(u, v):
        prev = {u: None}
        q = deque([u])
        while q:
            x = q.popleft()
            if x == v:
                break
            for y in adj[x]:
                if y not in prev:
                    prev[y] = x
                    q.append(y)
        path = [v]
        while prev[path[-1]] is not None:
            path.append(prev[path[-1]])
        return path[::-1]

    pairs = list(combinations(range(NJ), 2))  # 210
    c_np = np.zeros((len(pairs), NJ), np.float32)
    t_np = np.zeros((len(pairs), NJ), np.float32)
    for pi, (u, v) in enumerate(pairs):
        pa = bfs_path(u, v)
        for m in range(len(pa) - 1):
            c_np[pi, pa[m]] += 1.0 if par[pa[m]] == pa[m + 1] else -1.0
        t_np[pi, u] += 1.0
        t_np[pi, v] -= 1.0

    # W[t*63 + j*3 + c, p*3 + c] = C[p,j] (t=0) / -T[p,j] (t=1)
    eye3 = np.eye(3, dtype=np.float32)
    w_in = np.einsum('pj,cd->jcpd', c_np, eye3).reshape(63, 630)
    w_tg = np.einsum('pj,cd->jcpd', -t_np, eye3).reshape(63, 630)
    return np.ascontiguousarray(np.concatenate([w_in, w_tg], axis=0))  # [126, 630]


_W = _build_w()


def _build_w_padded():
    """[126, 640] bf16: chunk c of W's columns at [128c, 128c+126), zero pad."""
    import ml_dtypes
    wp = np.zeros((126, N_CCH * 128), np.float32)
    for c in range(N_CCH):
        wp[:, 128 * c:128 * c + 126] = _W[:, 126 * c:126 * (c + 1)]
    return wp.astype(ml_dtypes.bfloat16)


# ---------------------------------------------------------------- bass build
_NC = None

# tuning knobs (A/B'd on hardware).  Winning config: fd=1024 x 4 PSUM bufs
# decouples matmul fills from the ACT/DVE PSUM reads (fd=2048 x 2 bufs lost
# ~6 us/rep to fill/reduce serialization); the kernel then sits on the
# PSUM-read wall itself (~22 us/rep, the no-matmul probe measures the same).
CONFIG = {
    "act_ns": 997.0,    # sustained ns per [128, FD] abs-reduce tile on ACT
    "dve_ns": 1192.0,   # sustained ns per tile on DVE
    "split": "auto",    # "auto" greedy by the ns above | "act" | "dve"
    "n_act": 20,        # int: exactly n_act of the tiles go to ACT (Bresenham
                        # interleaved with DVE); overrides "split".  HW sweep
                        # at fd=1024 (40 tiles): 18->18.9us 20->17.3 22->20.5
                        # 24->24.8 -- even 20/20 split wins
    "psum_bf16": False,  # matmul writes bf16 PSUM (rejected by bass: fp32 only)
    "psum_bufs": 4,
    "fd": 1024,         # PSUM tile free dim (2 banks)
}


def _build_bass(n_reps=1):
    import concourse.bacc as bacc
    import concourse.mybir as mybir
    import concourse.tile as tile

    f32 = mybir.dt.float32
    bf16 = mybir.dt.bfloat16
    nc = bacc.Bacc("TRN2", target_bir_lowering=False, debug=False)

    zt = nc.dram_tensor("zt", [126, B_CORE], bf16, kind="ExternalInput")
    out = nc.dram_tensor("out", [128, 1], f32, kind="ExternalOutput")

    w_dram = nc.inline_tensor(_build_w_padded(), name="w_const")

    psum_dt = bf16 if CONFIG["psum_bf16"] else f32
    mm_n = 1024 if CONFIG["psum_bf16"] else 512  # one PSUM bank per matmul
    FD = CONFIG["fd"]
    N_WIN = B_CORE // FD
    n_tiles = N_WIN * N_CCH

    with tile.TileContext(nc) as tc:
        with (
            tc.tile_pool(name="consts", bufs=1) as consts,
            tc.tile_pool(name="ztp", bufs=2) as zt_pool,
            tc.tile_pool(name="staged", bufs=2) as staged_pool,
            tc.tile_pool(name="psumD", bufs=CONFIG["psum_bufs"],
                         space="PSUM") as psum_pool,
            tc.tile_pool(name="misc", bufs=1) as misc,
        ):
            w_sb = consts.tile([126, N_CCH * 128], bf16)
            nc.sync.dma_start(w_sb[:], w_dram[:])

            scratch = misc.tile([128, FD], psum_dt)  # ACT abs dump (never read)

            # greedy ACT/DVE balance (sustained-ns estimates)
            eng_t = {"act": 0.0, "dve": 0.0, "i": 0}

            def emit_absred(col, dps):
                act_ns, dve_ns = CONFIG["act_ns"], CONFIG["dve_ns"]
                i = eng_t["i"] % n_tiles
                eng_t["i"] += 1
                if CONFIG["n_act"] is not None:
                    na = CONFIG["n_act"]
                    use_act = ((i + 1) * na) // n_tiles > (i * na) // n_tiles
                elif CONFIG["split"] == "act":
                    use_act = True
                elif CONFIG["split"] == "dve":
                    use_act = False
                else:
                    use_act = eng_t["act"] + act_ns <= eng_t["dve"] + dve_ns
                if use_act:
                    eng_t["act"] += act_ns
                    nc.scalar.activation(
                        scratch[:], dps[:],
                        mybir.ActivationFunctionType.Abs, accum_out=col)
                else:
                    eng_t["dve"] += dve_ns
                    nc.vector.tensor_reduce(
                        col, dps[:], axis=mybir.AxisListType.X,
                        op=mybir.AluOpType.add, apply_absolute_value=True)

            for rep in range(n_reps):
                zt_sb = zt_pool.tile([126, B_CORE], bf16)
                nc.sync.dma_start(zt_sb[:], zt[:])

                acc = staged_pool.tile([128, N_WIN * N_CCH], f32, tag="acc")
                final = staged_pool.tile([128, 1], f32, tag="final")

                for w in range(N_WIN):
                    for c in range(N_CCH):
                        dps = psum_pool.tile([128, FD], psum_dt)
                        for q in range(FD // mm_n):
                            nc.tensor.matmul(
                                dps[:, q * mm_n:(q + 1) * mm_n],
                                w_sb[:, 128 * c:128 * c + 128],
                                zt_sb[:, w * FD + q * mm_n:w * FD + (q + 1) * mm_n])
                        i = w * N_CCH + c
                        emit_absred(acc[:, i:i + 1], dps)

                nc.vector.tensor_reduce(
                    final[:], acc[:], axis=mybir.AxisListType.X,
                    op=mybir.AluOpType.add)
                nc.sync.dma_start(out[:], final[:])

    nc.compile()
    return nc


# ---------------------------------------------------------------- host side
def _make_in_maps(inp, tgt):
    import ml_dtypes
    z = np.concatenate(
        [np.asarray(inp, np.float32), np.asarray(tgt, np.float32)], axis=1)
    in_maps = []
    for i in range(N_CORES):
        sl = slice(i * B_CORE, (i + 1) * B_CORE)
        zt_i = np.ascontiguousarray(z[sl].T).astype(ml_dtypes.bfloat16)
        in_maps.append({"zt": zt_i})
    return in_maps


def kernel(input, target):
    global _NC
    from concourse.bass_utils import run_bass_kernel_spmd

    if _NC is None:
        _NC = _build_bass()

    assert input.shape == (B_FULL, NJ * 3) and target.shape == (B_FULL, NJ * 3)
    in_maps = _make_in_maps(input, target)

    res = run_bass_kernel_spmd(_NC, in_maps, core_ids=list(range(N_CORES)))
    total = np.float64(0.0)
    for r in res.results:
        total += np.float64(r["out"].astype(np.float64).sum())
    return np.array([total / B_FULL], dtype=np.float32)
